# revision 1
# baseline (speedup 1.0000x reference)
"""Trainium2 Bass kernel for CardAwarePolicy (counts-reformulated MHA + folded MLPs).

Self-contained: takes full unsharded inputs, shards batch across 8 NeuronCores
(pure data parallel), runs a Tile/Bass kernel per core, gathers the output.

Math summary (per batch element, validated against the reference in numpy):
  The masked 4-head self-attention over the 8 hand slots depends on the hand
  only through its card-count vector n[c] (c in 0..53), because
  sum_q sum_k attn[h,q,k] v[k] collapses to card-vocabulary sums once the
  softmax exp() is folded into a precomputed table EG0[h,c',c] (stabilized,
  with the pad-card column zeroed).  With Nsc = n * (1/hand_len):
      den  = EG0 @ Nsc                  (per head, 54 query-cards)
      T    = Nrep / den                 (Nrep = Nsc stacked per head-pair)
      W2   = EG0^T @ T
      Y    = W2 * Nrep
      hand-term of ctx1 = BIG @ Y       (BIG folds V-table and out_w/ctx_w1)
  All other branches (enemy embed, game-state/discard MLPs, ctx MLP, action
  scorer) are folded into matmuls with the tiny weights pre-combined on host.
"""

import sys
import numpy as np

sys.path.insert(0, "/opt/trn_rl_repo")

B_FULL = 65536
N_CORES = 8
BC = B_FULL // N_CORES        # 8192 per core
TN = 512                      # batch columns per tile (= matmul free dim)
NT = BC // TN                 # 16 tiles per core
NH, HD, E, HS, A = 4, 3, 12, 8, 20

_CACHE = {}


# ---------------------------------------------------------------- host folding
def _fold_tables(inp):
    f = lambda k: np.asarray(inp[k], np.float64)
    card_emb, enemy_emb = f("card_emb"), f("enemy_emb")
    in_w, in_b = f("in_w"), f("in_b")
    out_w, out_b = f("out_w"), f("out_b")
    gs_w1, gs_b1, gs_w2, gs_b2 = f("gs_w1"), f("gs_b1"), f("gs_w2"), f("gs_b2")
    dp_w1, dp_b1, dp_w2, dp_b2 = f("dp_w1"), f("dp_b1"), f("dp_w2"), f("dp_b2")
    ctx_w1, ctx_b1, ctx_w2, ctx_b2 = f("ctx_w1"), f("ctx_b1"), f("ctx_w2"), f("ctx_b2")
    sc_w1, sc_b1, sc_w2, sc_b2 = f("sc_w1"), f("sc_b1"), f("sc_w2"), f("sc_b2")
    aci = np.asarray(inp["action_card_indices"])
    nva = int(inp["num_valid_actions"])

    Tq = card_emb @ in_w[0:12].T + in_b[0:12]
    Tk = card_emb @ in_w[12:24].T + in_b[12:24]
    Tv = card_emb @ in_w[24:36].T + in_b[24:36]
    G = np.zeros((NH, 54, 54))
    for h in range(NH):
        G[h] = (Tq[:, 3 * h:3 * h + 3] @ Tk[:, 3 * h:3 * h + 3].T) / np.sqrt(HD)
    EG0 = np.exp(G - G.max(axis=2, keepdims=True))
    EG0[:, :, 0] = 0.0

    T = {}

    def den_lhsT(heads):
        out = np.zeros((54, 108))
        for j, h in enumerate(heads):
            out[:, 54 * j:54 * j + 54] = EG0[h].T
        return out

    def w2_lhsT(heads):
        out = np.zeros((108, 108))
        for j, h in enumerate(heads):
            out[54 * j:54 * j + 54, 54 * j:54 * j + 54] = EG0[h]
        return out

    W1hh = ctx_w1[:, 0:12] @ out_w
    u0 = 8.0 * (ctx_w1[:, 0:12] @ out_b)

    def big_lhsT(heads, with_u0):
        out = np.zeros((109 if with_u0 else 108, 128))
        for j, h in enumerate(heads):
            out[54 * j:54 * j + 54, :] = Tv[:, 3 * h:3 * h + 3] @ W1hh[:, 3 * h:3 * h + 3].T
        if with_u0:
            out[108, :] = u0
        return out

    T["t_denA"], T["t_denB"] = den_lhsT((0, 1)), den_lhsT((2, 3))
    T["t_w2A"], T["t_w2B"] = w2_lhsT((0, 1)), w2_lhsT((2, 3))
    bigA = np.zeros((121, 128))
    bigA[0:108] = big_lhsT((0, 1), False)
    T["t_bigB"] = big_lhsT((2, 3), False)

    t_gd = np.zeros((66, 128))
    t_gd[0:12, 0:64] = gs_w1.T
    t_gd[12:66, 64:128] = dp_w1.T
    T["t_gd"] = t_gd
    T["b_gd"] = np.concatenate([gs_b1, dp_b1])[:, None]

    t_agd = np.zeros((128, 128))
    t_agd[0:64, :] = (ctx_w1[:, 24:30] @ gs_w2).T
    t_agd[64:128, :] = (ctx_w1[:, 30:36] @ dp_w2).T
    T["t_agd"] = t_agd

    bias_ctx1 = ctx_b1 + ctx_w1[:, 24:30] @ gs_b2 + ctx_w1[:, 30:36] @ dp_b2
    T["b_ctx1"] = bias_ctx1[:, None]
    bigA[108:120, :] = ctx_w1[:, 12:24].T
    bigA[120, :] = u0
    T["t_bigA"] = bigA

    W_uc = sc_w1[:, 0:128] @ ctx_w2
    t_uc4 = np.zeros((128, 128))
    for a in range(4):
        t_uc4[:, 32 * a:32 * a + 32] = W_uc.T
    T["t_uc4"] = t_uc4

    am = (aci != 0).astype(np.float64)
    cnt = np.maximum(am.sum(axis=1), 1.0)
    arep = (card_emb[aci] * am[:, :, None]).sum(axis=1) / cnt[:, None]
    v = arep @ sc_w1[:, 128:140].T + sc_b1 + sc_w1[:, 0:128] @ ctx_b2  # [20,32]
    b_H = np.zeros((128, 5))
    for g in range(5):
        for a in range(4):
            b_H[32 * a:32 * a + 32, g] = v[4 * g + a]
    T["b_H"] = b_H

    for g in range(5):
        t = np.zeros((128, 20))
        for a in range(4):
            t[32 * a:32 * a + 32, 4 * g + a] = sc_w2[0]
        T[f"t_sc{g}"] = t

    b_out = np.full((20, 1), float(np.asarray(sc_b2).reshape(-1)[0]))
    b_out[nva:] = -1e8
    T["b_out"] = b_out
    return {k: np.ascontiguousarray(v, np.float32) for k, v in T.items()}


# weight blob: each lhsT table occupies [rows, cols] at a column offset (base partition 0)
BLOB_LAYOUT = [  # name, rows, cols
    ("t_denA", 54, 108), ("t_denB", 54, 108),
    ("t_w2A", 108, 108), ("t_w2B", 108, 108),
    ("t_bigA", 121, 128), ("t_bigB", 108, 128),
    ("t_gd", 66, 128), ("t_agd", 128, 128), ("t_uc4", 128, 128),
    ("t_sc0", 128, 20), ("t_sc1", 128, 20), ("t_sc2", 128, 20),
    ("t_sc3", 128, 20), ("t_sc4", 128, 20),
]
BLOB_COLS = sum(c for _, _, c in BLOB_LAYOUT)
BIAS_LAYOUT = [("b_gd", 128, 1), ("b_H", 128, 5), ("b_ctx1", 128, 1), ("b_out", 20, 1)]
BIAS_COLS = sum(c for _, _, c in BIAS_LAYOUT)


def _pack_blobs(T):
    wb = np.zeros((128, BLOB_COLS), np.float32)
    off = 0
    for name, rows, cols in BLOB_LAYOUT:
        wb[0:rows, off:off + cols] = T[name]
        off += cols
    bb = np.zeros((128, BIAS_COLS), np.float32)
    off = 0
    for name, rows, cols in BIAS_LAYOUT:
        bb[0:rows, off:off + cols] = T[name]
        off += cols
    return wb, bb


# ---------------------------------------------------------------- bass module
def _build_module(bc):
    import concourse.bass as bass
    import concourse.bacc as bacc
    import concourse.mybir as mybir
    from concourse import tile

    dt = mybir.dt
    f32, f32r, i16 = dt.float32, dt.float32r, dt.int16
    nt = bc // TN

    nc = bacc.Bacc("TRN2", target_bir_lowering=False, debug=False)

    din = lambda name, shape, dtype: nc.dram_tensor(name, list(shape), dtype, kind="ExternalInput").ap()
    wb_d = din("wblob", (128, BLOB_COLS), f32r)
    bb_d = din("bblob", (128, BIAS_COLS), f32)
    x66_d = din("x66", (66, bc), f32r)
    exu_d = din("exu", (13, bc), f32r)
    nsc_d = din("nsc", (nt, 108, TN), f32r)
    out_d = nc.dram_tensor("out", [20, bc], f32, kind="ExternalOutput").ap()

    with tile.TileContext(nc) as tc:
        with (
            tc.tile_pool(name="const", bufs=1) as cpool,
            tc.tile_pool(name="io", bufs=3) as io,
            tc.tile_pool(name="work", bufs=2) as wk,
            tc.tile_pool(name="ps", bufs=1, space="PSUM") as ps,
        ):
            wblob = cpool.tile([128, BLOB_COLS], f32r, name="wblob")
            nc.sync.dma_start(out=wblob, in_=wb_d)
            bblob = cpool.tile([128, BIAS_COLS], f32, name="bblob")
            nc.sync.dma_start(out=bblob, in_=bb_d)
            tb = {}
            off = 0
            for name, rows, cols in BLOB_LAYOUT:
                tb[name] = wblob[0:rows, off:off + cols]
                off += cols
            boff = 0
            for name, rows, cols in BIAS_LAYOUT:
                tb[name] = bblob[0:rows, boff:boff + cols]
                boff += cols

            for t in range(nt):
                col = slice(t * TN, (t + 1) * TN)
                nsc2 = io.tile([108, TN], f32r, tag="nsc", bufs=3, name=f"nsc_{t}")
                nc.sync.dma_start(out=nsc2, in_=nsc_d[t])

                x_t = io.tile([66, TN], f32r, tag="x", name=f"x_{t}")
                nc.scalar.dma_start(out=x_t, in_=x66_d[:, col])

                # --- hand branch (counts formulation) ---
                denA_ps = ps.tile([108, TN], f32, tag="denA", name=f"denA_{t}")
                nc.tensor.matmul(denA_ps, tb["t_denA"], nsc2[0:54, :], start=True, stop=True)
                denB_ps = ps.tile([108, TN], f32, tag="denB", name=f"denB_{t}")
                nc.tensor.matmul(denB_ps, tb["t_denB"], nsc2[0:54, :], start=True, stop=True)

                rdA = wk.tile([108, TN], f32, tag="rdA", name=f"rdA_{t}")
                nc.vector.reciprocal_approx_fast(out=rdA, in_=denA_ps)
                rdB = wk.tile([108, TN], f32, tag="rdB", name=f"rdB_{t}")
                nc.vector.reciprocal_approx_fast(out=rdB, in_=denB_ps)
                TA = wk.tile([108, TN], f32r, tag="TA", name=f"TA_{t}")
                nc.gpsimd.tensor_tensor(TA, nsc2.bitcast(f32), rdA, mybir.AluOpType.mult)
                TB = wk.tile([108, TN], f32r, tag="TB", name=f"TB_{t}")
                nc.gpsimd.tensor_tensor(TB, nsc2.bitcast(f32), rdB, mybir.AluOpType.mult)

                w2A_ps = ps.tile([108, TN], f32, tag="w2A", name=f"w2A_{t}")
                nc.tensor.matmul(w2A_ps, tb["t_w2A"], TA, start=True, stop=True)
                w2B_ps = ps.tile([108, TN], f32, tag="w2B", name=f"w2B_{t}")
                nc.tensor.matmul(w2B_ps, tb["t_w2B"], TB, start=True, stop=True)

                YA = wk.tile([121, TN], f32r, tag="YA", name=f"YA_{t}")
                nc.sync.dma_start(out=YA[108:121, :], in_=exu_d[:, col])
                nc.vector.tensor_tensor(YA[0:108, :], w2A_ps, nsc2.bitcast(f32), mybir.AluOpType.mult)
                YB = wk.tile([108, TN], f32r, tag="YB", name=f"YB_{t}")
                nc.vector.tensor_tensor(YB, w2B_ps, nsc2.bitcast(f32), mybir.AluOpType.mult)

                # --- game-state / discard encoders ---
                gd1_ps = ps.tile([128, TN], f32, tag="mid", bufs=2, name=f"gd1_{t}")
                nc.tensor.matmul(gd1_ps, tb["t_gd"], x_t, start=True, stop=True)
                gd1r = wk.tile([128, TN], f32r, tag="gd1r", name=f"gd1r_{t}")
                nc.vector.tensor_scalar(gd1r, gd1_ps, tb["b_gd"], 0.0,
                                        mybir.AluOpType.add, mybir.AluOpType.max)

                # --- ctx layer 1 accumulation ---
                ctx1_ps = ps.tile([128, TN], f32, tag="ctx1", name=f"ctx1_{t}")
                nc.tensor.matmul(ctx1_ps, tb["t_bigA"], YA, start=True, stop=False)
                nc.tensor.matmul(ctx1_ps, tb["t_bigB"], YB, start=False, stop=False)
                nc.tensor.matmul(ctx1_ps, tb["t_agd"], gd1r, start=False, stop=True)

                ctx1 = wk.tile([128, TN], f32r, tag="ctx1s", name=f"ctx1s_{t}")
                nc.scalar.activation(ctx1, ctx1_ps, mybir.ActivationFunctionType.Relu,
                                     bias=tb["b_ctx1"], scale=1.0)

                # --- scorer ---
                u4_ps = ps.tile([128, TN], f32, tag="mid", bufs=2, name=f"u4_{t}")
                nc.tensor.matmul(u4_ps, tb["t_uc4"], ctx1, start=True, stop=True)

                sc_ps = ps.tile([20, TN], f32, tag="sc", name=f"sc_{t}")
                for g in range(5):
                    H = wk.tile([128, TN], f32r, tag=f"H{g}", name=f"H{g}_{t}")
                    if g < 1:
                        nc.vector.tensor_scalar(H, u4_ps, tb["b_H"][:, g:g + 1], 0.0,
                                                mybir.AluOpType.add, mybir.AluOpType.max)
                    else:
                        nc.scalar.activation(H, u4_ps, mybir.ActivationFunctionType.Relu,
                                             bias=tb["b_H"][:, g:g + 1], scale=1.0)
                    nc.tensor.matmul(sc_ps, tb[f"t_sc{g}"], H,
                                     start=(g == 0), stop=(g == 4))

                sc_sb = wk.tile([20, TN], f32, tag="scsb", name=f"scsb_{t}")
                nc.scalar.activation(sc_sb, sc_ps, mybir.ActivationFunctionType.Identity,
                                     bias=tb["b_out"], scale=1.0)
                nc.scalar.dma_start(out=out_d[:, col], in_=sc_sb)

    nc.finalize()
    return nc


def _get_module(bc=BC):
    key = ("mod", bc)
    if key not in _CACHE:
        _CACHE[key] = _build_module(bc)
    return _CACHE[key]


# ---------------------------------------------------------------- host prep
def _prep_data(inp):
    """Full-batch host prep: counts, scaling, layout. Returns per-core input maps."""
    hc = np.asarray(inp["hand_cards"])
    B = hc.shape[0]
    gs = np.asarray(inp["game_state"], np.float32)
    dp = np.asarray(inp["discard_pile_cards"], np.float32)
    en = np.asarray(inp["enemy_card"]).reshape(B).astype(np.int64)
    hsz = np.asarray(inp["hand_size"]).astype(np.float64)

    idx = (hc.astype(np.int64) + 54 * np.arange(B, dtype=np.int64)[:, None]).ravel()
    counts = np.bincount(idx, minlength=B * 54).reshape(B, 54)
    rlen = (1.0 / np.maximum(hsz, 1.0)).astype(np.float32)
    nsc = (counts.astype(np.float32) * rlen[:, None]).T  # [54, B]

    x66 = np.empty((66, B), np.float32)
    x66[0:12] = gs.T
    x66[12:66] = dp.T
    en_emb = np.asarray(inp["enemy_emb"], np.float32)
    exu = np.empty((13, B), np.float32)
    exu[0:12] = en_emb[en].T
    exu[12] = rlen

    tables = _fold_tables(inp)
    wb, bb = _pack_blobs(tables)

    maps = []
    for c in range(N_CORES):
        cols = slice(c * BC, (c + 1) * BC)
        nsc_c = np.ascontiguousarray(nsc[:, cols])           # [54, BC]
        nsc_p = nsc_c.reshape(54, NT, TN).transpose(1, 0, 2)
        nsc_p = np.ascontiguousarray(np.concatenate([nsc_p, nsc_p], axis=1))  # [NT,108,TN]
        m = {"wblob": wb, "bblob": bb,
             "x66": np.ascontiguousarray(x66[:, cols]),
             "exu": np.ascontiguousarray(exu[:, cols]),
             "nsc": nsc_p}
        maps.append(m)
    return maps


# ---------------------------------------------------------------- entry points
def _enable_ldw_opt():
    # Dedup/pipeline PE weight loads: ~160ns x 224 LDWEIGHTS per run otherwise.
    import concourse.bass_utils as bu
    if getattr(bu, "_ldw_opt_patched", False):
        return
    orig = bu.run_command

    def patched(argv, **kw):
        argv = [a.replace("--enable-ldw-opt=false", "--enable-ldw-opt=true")
                if isinstance(a, str) else a for a in argv]
        return orig(argv, **kw)

    bu.run_command = patched
    bu._ldw_opt_patched = True


def _run(inputs, trace=False):
    from concourse.bass_utils import run_bass_kernel_spmd
    _enable_ldw_opt()

    in_maps = _prep_data(inputs)
    nc = _get_module()
    res = run_bass_kernel_spmd(nc, in_maps, list(range(N_CORES)), trace=trace)
    out = np.concatenate([r["out"] for r in res.results], axis=1).T  # [B, 20]
    return np.ascontiguousarray(out), res


def kernel(**inputs) -> np.ndarray:
    out, _ = _run(inputs, trace=False)
    return out

